# revision 1
# baseline (speedup 1.0000x reference)
"""LogSparseAttention Trainium2 kernel (8-core SPMD).

Sharding: 8 cores = 2 batches x 4 head-groups (4 heads = 256 channels each).
Each core: causal convs (q, k) for its 256 output channels over the full-D
input, v projection, local window-16 attention + 8 exponential-jump terms for
its 4 heads, then a partial output projection over its 256 channels.
Host sums the 8 partial [D, T] outputs (4 per batch) and adds p_b.

Layouts on chip (partition dim first):
  xT      [1024, 15+2048] fp32r   x transposed, left-padded 15 zeros
  q_sb    2 x [128, 2048]         conv out (+bias, x1/8), channel-partition
  k_sb    2 x [128, 15+2048]      conv out (+bias), left-padded zeros
  v_dt    2 x [128, 2048] f32     v (+bias), channel-partition (jump FMA)
  v_td    17 x [128, 4, 65] f32r  v (no bias) token-partition, +15-shifted,
                                  per-head ones column (Z accumulator)
  z       2 x [128, 2048] f32     attention output accumulator [d, t]
"""
import sys

sys.path.insert(0, "/opt/trn_rl_repo")

import numpy as np
import concourse.bass as bass
import concourse.bacc as bacc
import concourse.tile as tile
from concourse import mybir

f32 = mybir.dt.float32
f32r = mybir.dt.float32r
AL = mybir.AluOpType
AF = mybir.ActivationFunctionType

B, T, D = 2, 2048, 1024
H, W, E = 16, 16, 8
HD = D // H                  # 64
NCORES = 8
HPC = 4                      # heads per core
CH = HPC * HD                # 256 channels per core
NG = 2                       # head-pairs per core, 128 channels each
TP = W - 1                   # 15 (left pad)
TPAD = T + TP                # 2063
NT128 = T // 128             # 16
NT256 = T // 256             # 8
NT512 = T // 512             # 4
KT = D // 128                # 8 k-tiles over the input dim
SCALE = 1.0 / float(np.sqrt(HD))
MASKVAL = -200.0

_CACHE = {}


def build_program():
    if "nc" in _CACHE:
        return _CACHE["nc"]
    import contextlib
    nc = bacc.Bacc()

    xT = nc.dram_tensor("xT", [D, TPAD], f32r, kind="ExternalInput")
    qw = nc.dram_tensor("qw", [W, D, CH], f32r, kind="ExternalInput")
    kw = nc.dram_tensor("kw", [W, D, CH], f32r, kind="ExternalInput")
    vw = nc.dram_tensor("vw", [D, CH], f32r, kind="ExternalInput")
    pw = nc.dram_tensor("pw", [CH, D], f32r, kind="ExternalInput")
    qb = nc.dram_tensor("qb", [CH, 1], f32, kind="ExternalInput")
    kb = nc.dram_tensor("kb", [CH, 1], f32, kind="ExternalInput")
    vb = nc.dram_tensor("vb", [CH, 1], f32, kind="ExternalInput")
    mask = nc.dram_tensor("mask", [272, 256], f32, kind="ExternalInput")
    ident = nc.dram_tensor("ident", [128, 128], f32r, kind="ExternalInput")
    onesp = nc.dram_tensor("onesp", [E, 128, 2 * E], f32r, kind="ExternalInput")
    ones4 = nc.dram_tensor("ones4", [128, 2 * HPC], f32r, kind="ExternalInput")
    zpad = nc.dram_tensor("zpad", [128, TP], f32r, kind="ExternalInput")
    vbrow = nc.dram_tensor("vbrow", [1, CH], f32, kind="ExternalInput")
    vzero = nc.dram_tensor("vzero", [TP, CH], f32r, kind="ExternalInput")
    y = nc.dram_tensor("y", [D, T], f32, kind="ExternalOutput")
    import os as _os
    DBG = bool(_os.environ.get("KERNEL_DEBUG"))
    if DBG:
        dbg_q = nc.dram_tensor("dbg_q", [CH, T], f32, kind="ExternalOutput")
        dbg_k = nc.dram_tensor("dbg_k", [CH, TPAD], f32, kind="ExternalOutput")
        dbg_v = nc.dram_tensor("dbg_v", [CH, T], f32, kind="ExternalOutput")
        dbg_z = nc.dram_tensor("dbg_z", [CH, T], f32, kind="ExternalOutput")
        dbg_a = nc.dram_tensor("dbg_a", [NG, 2 * E, T], f32, kind="ExternalOutput")
    alpha_d = [nc.dram_tensor(f"alpha_d{g}", [2 * E, T], f32) for g in range(NG)]

    with tile.TileContext(nc) as tc:
        with contextlib.ExitStack() as ctx:
            consts = ctx.enter_context(tc.tile_pool(name="consts", bufs=1))
            main = ctx.enter_context(tc.tile_pool(name="main", bufs=1))

            # ---- constants ----
            m0 = consts.tile([128, 256], f32)
            m1 = consts.tile([128, 256], f32)
            m2 = consts.tile([TP, 256], f32)
            nc.sync.dma_start(m0[:], mask[0:128, :])
            nc.sync.dma_start(m1[:], mask[128:256, :])
            nc.sync.dma_start(m2[:], mask[256:271, :])
            id_sb = consts.tile([128, 128], f32r)
            nc.sync.dma_start(id_sb[:], ident[:])
            onesp_sb = consts.tile([128, E, 2 * E], f32r)
            nc.sync.dma_start(onesp_sb[:], onesp.rearrange("e p m -> p e m"))
            qb_sb = consts.tile([128, NG], f32)
            kb_sb = consts.tile([128, NG], f32)
            vb_sb = consts.tile([128, NG], f32)
            nc.sync.dma_start(qb_sb[:], qb.rearrange("(g p) o -> p (g o)", g=NG))
            nc.sync.dma_start(kb_sb[:], kb.rearrange("(g p) o -> p (g o)", g=NG))
            nc.sync.dma_start(vb_sb[:], vb.rearrange("(g p) o -> p (g o)", g=NG))
            pw_sb = [consts.tile([128, D], f32r, tag=f"pw{g}", name=f"pw_sb{g}") for g in range(NG)]
            for g in range(NG):
                nc.sync.dma_start(pw_sb[g][:], pw[128 * g:128 * (g + 1), :])
            vw_sb = [consts.tile([128, CH], f32r, tag=f"vw{i}", name=f"vw_sb{i}") for i in range(KT)]
            for i in range(KT):
                nc.sync.dma_start(vw_sb[i][:], vw[128 * i:128 * (i + 1), :])
            vbt = consts.tile([128, CH], f32)
            nc.sync.dma_start(vbt[:], vbrow[:].to_broadcast((128, CH)))

            # ---- persistent activations ----
            q_sb = [main.tile([128, T], f32r, tag=f"q{g}", name=f"q_sb{g}") for g in range(NG)]
            k_sb = [main.tile([128, TPAD], f32r, tag=f"k{g}", name=f"k_sb{g}") for g in range(NG)]
            v_dt = [main.tile([128, T], f32, tag=f"vdt{g}", name=f"v_dt{g}") for g in range(NG)]
            v_td = [main.tile([128, HPC, HD + 2], f32r, tag=f"vtd{j}", name=f"v_td{j}")
                    for j in range(NT128 + 1)]
            z = [main.tile([128, T], f32, tag=f"z{g}", name=f"z{g}") for g in range(NG)]

            for g in range(NG):
                nc.sync.dma_start(k_sb[g][:, 0:TP], zpad[:])

            # ================= phase 1: x-resident matmuls =================
            with tc.tile_pool(name="xw", bufs=1) as xpool, \
                 tc.tile_pool(name="wstream", bufs=6) as wpool, \
                 tc.tile_pool(name="psA", bufs=1, space="PSUM") as psA:
                xT_sb = [xpool.tile([128, TPAD], f32r, tag=f"x{i}", name=f"xT_sb{i}") for i in range(KT)]
                for i in range(KT):
                    nc.sync.dma_start(xT_sb[i][:], xT[128 * i:128 * (i + 1), :])

                # v in [t, d] layout (Form M=t), shifted +15, no bias
                for j in range(NT128 + 1):
                    mrow = 128 if j < NT128 else TP
                    pv = psA.tile([128, 512], f32, tag=f"bank{j % 2}", bufs=1, name=f"pv{j}")[:, 0:CH]
                    for i in range(KT):
                        nc.tensor.matmul(
                            pv[0:mrow, :],
                            xT_sb[i][:, 128 * j:128 * j + mrow],
                            vw_sb[i][:],
                            start=(i == 0), stop=(i == KT - 1),
                        )
                    nc.vector.tensor_tensor(
                        v_td[j][0:mrow, :, 0:HD],
                        pv[0:mrow, :].rearrange("p (h d) -> p h d", h=HPC),
                        vbt[0:mrow, :].rearrange("p (h d) -> p h d", h=HPC),
                        AL.add,
                    )
                    if j == 0:
                        # keys at t<0 are zero-padded AFTER bias in the reference
                        nc.sync.dma_start(
                            v_td[0][0:TP, :, 0:HD],
                            vzero.rearrange("p (h d) -> p h d", h=HPC))
                    nc.sync.dma_start(
                        v_td[j][:, :, HD:HD + 2],
                        ones4.rearrange("p (h o) -> p h o", o=2))

                # v in [o, t] layout (Form M=o), with bias
                for g in range(NG):
                    for t4 in range(NT512):
                        pv2 = psA.tile([128, 512], f32, tag=f"bank{2 + t4 % 2}", bufs=1, name=f"pv2_{g}_{t4}")
                        for i in range(KT):
                            nc.tensor.matmul(
                                pv2[:],
                                vw_sb[i][:, 128 * g:128 * (g + 1)],
                                xT_sb[i][:, TP + 512 * t4: TP + 512 * (t4 + 1)],
                                start=(i == 0), stop=(i == KT - 1),
                            )
                        nc.vector.tensor_scalar(
                            v_dt[g][:, 512 * t4:512 * (t4 + 1)], pv2[:],
                            vb_sb[:, g:g + 1], None, op0=AL.add,
                        )

                # q/k causal convs: 8 open psum groups, (dt, i) outer
                for (wdram, dst, bias_sb, is_q) in (
                    (qw, q_sb, qb_sb, True),
                    (kw, k_sb, kb_sb, False),
                ):
                    pc = [psA.tile([128, 512], f32, tag=f"bank{o2 * NT512 + t4}", bufs=1, name=f"pc{o2}_{t4}")
                          for o2 in range(NG) for t4 in range(NT512)]
                    for dt in range(W):
                        for i in range(KT):
                            wt = wpool.tile([128, CH], f32r, tag="w")
                            nc.sync.dma_start(
                                wt[:], wdram[dt, 128 * i:128 * (i + 1), :])
                            first = (dt == 0 and i == 0)
                            last = (dt == W - 1 and i == KT - 1)
                            for o2 in range(NG):
                                for t4 in range(NT512):
                                    nc.tensor.matmul(
                                        pc[o2 * NT512 + t4][:],
                                        wt[:, 128 * o2:128 * (o2 + 1)],
                                        xT_sb[i][:, 512 * t4 + dt:512 * t4 + dt + 512],
                                        start=first, stop=last,
                                    )
                    for o2 in range(NG):
                        for t4 in range(NT512):
                            p = pc[o2 * NT512 + t4]
                            if is_q:
                                nc.vector.tensor_scalar(
                                    dst[o2][:, 512 * t4:512 * (t4 + 1)], p[:],
                                    bias_sb[:, o2:o2 + 1], SCALE,
                                    op0=AL.add, op1=AL.mult,
                                )
                            else:
                                nc.vector.tensor_scalar(
                                    dst[o2][:, TP + 512 * t4:TP + 512 * (t4 + 1)],
                                    p[:], bias_sb[:, o2:o2 + 1], None, op0=AL.add,
                                )

            # ============ phases 2-4: attention + output projection ============
            # PSUM budget (8 banks): score x2, outu/tp shared x2, js(16x2048) x4
            with tc.tile_pool(name="attn", bufs=1) as apool:
              with tc.tile_pool(name="psATT", bufs=1, space="PSUM") as psL:
                def local_attn(g):
                    for c in range(NT256):
                        exps = []
                        for hh in range(2):
                            r0, r1 = 64 * hh, 64 * hh + 64
                            e0 = apool.tile([128, 256], f32r, tag="e0", bufs=3, name="e0")
                            e1 = apool.tile([128, 256], f32r, tag="e1", bufs=3, name="e1")
                            e2 = apool.tile([TP, 256], f32r, tag="e2", bufs=3, name="e2")
                            for (et, msk, s0, srows) in (
                                (e0, m0, 256 * c, 128),
                                (e1, m1, 256 * c + 128, 128),
                                (e2, m2, 256 * c + 256, TP),
                            ):
                                ps = psL.tile([128, 256], f32, tag="score", bufs=2, name="ps")
                                nc.tensor.matmul(
                                    ps[0:srows, :],
                                    k_sb[g][r0:r1, s0:s0 + srows],
                                    q_sb[g][r0:r1, 256 * c:256 * (c + 1)],
                                    start=True, stop=True,
                                )
                                nc.vector.tensor_tensor(
                                    ps[0:srows, :], ps[0:srows, :],
                                    msk[0:srows, :], AL.add)
                                nc.scalar.activation(
                                    et[0:srows, :], ps[0:srows, :], AF.Exp)
                            exps.append((e0, e1, e2))
                        for sub in range(2):
                            jj = 2 * c + sub
                            stage = apool.tile([128, 128], f32r, tag="stage", bufs=2, name="stage")
                            for hh in range(2):
                                e0, e1, e2 = exps[hh]
                                if sub == 0:
                                    lo, hi = e0[:, 0:128], e1[0:TP, 0:128]
                                else:
                                    lo, hi = e1[:, 128:256], e2[0:TP, 128:256]
                                hl = 2 * g + hh
                                po = psL.tile([128, HD + 2], f32, tag="tp", bufs=2, name="po")
                                nc.tensor.matmul(
                                    po[:], lo, v_td[jj][:, hl, :],
                                    start=True, stop=False,
                                )
                                nc.tensor.matmul(
                                    po[:], hi, v_td[jj + 1][0:TP, hl, :],
                                    start=False, stop=True,
                                )
                                rz = apool.tile([128, 1], f32, tag="rz", bufs=2, name="rz")
                                nc.vector.reciprocal(rz[:], po[:, HD:HD + 1])
                                nc.vector.tensor_scalar(
                                    stage[:, 64 * hh:64 * hh + 64],
                                    po[:, 0:HD], rz[:], None, op0=AL.mult,
                                )
                            pt = psL.tile([128, 128], f32r, tag="tp", bufs=2, name="pt")
                            nc.tensor.transpose(pt[:], stage[:], id_sb[:])
                            tcol = 256 * c + 128 * sub
                            nc.vector.tensor_copy(
                                z[g][:, tcol:tcol + 128], pt[:].bitcast(f32))

                def jump_scores(g):
                    pj = psL.tile([16, T], f32, tag="js", bufs=1, name="pj")
                    for e in range(E):
                        sh = 1 << e
                        tmp = apool.tile([128, T], f32r, tag="jtmp", bufs=2, name="jtmp")
                        nc.vector.tensor_tensor(
                            tmp[:, 0:T - sh],
                            q_sb[g][:, 0:T - sh].bitcast(f32),
                            k_sb[g][:, TP + sh:TP + T].bitcast(f32), AL.mult)
                        nc.vector.tensor_tensor(
                            tmp[:, T - sh:T],
                            q_sb[g][:, T - sh:T].bitcast(f32),
                            k_sb[g][:, TP:TP + sh].bitcast(f32), AL.mult)
                        for t4 in range(NT512):
                            nc.tensor.matmul(
                                pj[:, 512 * t4:512 * (t4 + 1)],
                                onesp_sb[:, e, :],
                                tmp[:, 512 * t4:512 * (t4 + 1)],
                                start=(e == 0), stop=(e == E - 1),
                            )
                    erows = apool.tile([16, T], f32, tag="erows", name="erows")
                    nc.scalar.activation(erows[:], pj[:], AF.Exp)
                    zsum = apool.tile([16, 1], f32, tag="zsum", name="zsum")
                    nc.vector.tensor_reduce(
                        zsum[:], erows[:], mybir.AxisListType.X, AL.add)
                    nc.vector.reciprocal(zsum[:], zsum[:])
                    arows = apool.tile([16, T], f32, tag="arows", name="arows")
                    nc.vector.tensor_scalar(
                        arows[:], erows[:], zsum[:], None, op0=AL.mult)
                    nc.sync.dma_start(alpha_d[g][:], arows[:])

                for g in range(NG):
                    local_attn(g)
                    jump_scores(g)

              def jump_fma(g):
                    for e in range(E):
                        sh = 1 << e
                        bc = apool.tile([128, T], f32, tag="bc", bufs=2, name="bc")
                        nc.sync.dma_start(
                            bc[0:64, :],
                            alpha_d[g][2 * e:2 * e + 1, :].to_broadcast((64, T)))
                        nc.sync.dma_start(
                            bc[64:128, :],
                            alpha_d[g][2 * e + 1:2 * e + 2, :].to_broadcast((64, T)))
                        ft = apool.tile([128, T], f32, tag="ft", bufs=2, name="ft")
                        # multiply on GpSimd (otherwise idle), accumulate via DMA CCE
                        nc.gpsimd.tensor_tensor(
                            ft[:, 0:T - sh], bc[:, 0:T - sh],
                            v_dt[g][:, sh:T], AL.mult)
                        nc.gpsimd.tensor_tensor(
                            ft[:, T - sh:T], bc[:, T - sh:T],
                            v_dt[g][:, 0:sh], AL.mult)
                        nc.gpsimd.dma_start(z[g][:], ft[:], accum_op=AL.add)

              for g in range(NG):
                    jump_fma(g)

              if DBG:
                for g in range(NG):
                    nc.sync.dma_start(dbg_q[128 * g:128 * (g + 1), :], q_sb[g][:].bitcast(f32))
                    nc.sync.dma_start(dbg_k[128 * g:128 * (g + 1), :], k_sb[g][:].bitcast(f32))
                    nc.sync.dma_start(dbg_v[128 * g:128 * (g + 1), :], v_dt[g][:])
                    nc.sync.dma_start(dbg_z[128 * g:128 * (g + 1), :], z[g][:])
                    nc.sync.dma_start(dbg_a[g], alpha_d[g][:])
              # ================= phase 4: output projection =================
              with tc.tile_pool(name="psL4", bufs=1, space="PSUM") as psL4:
                zr = [apool.tile([128, T], f32r, tag=f"zr{g}", name=f"zr{g}") for g in range(NG)]
                for g in range(NG):
                    nc.vector.tensor_copy(zr[g][:], z[g][:])
                for o8 in range(D // 128):
                    for t4 in range(NT512):
                        py = psL4.tile([128, 512], f32, tag="py", bufs=4, name="py")
                        for g in range(NG):
                            nc.tensor.matmul(
                                py[:],
                                pw_sb[g][:, 128 * o8:128 * (o8 + 1)],
                                zr[g][:, 512 * t4:512 * (t4 + 1)],
                                start=(g == 0), stop=(g == NG - 1),
                            )
                        ysb = apool.tile([128, 512], f32, tag="ysb", bufs=4, name="ysb")
                        nc.scalar.copy(ysb[:], py[:])
                        nc.sync.dma_start(
                            y[128 * o8:128 * (o8 + 1),
                              512 * t4:512 * (t4 + 1)], ysb[:])

    nc.compile()
    _CACHE["nc"] = nc
    return nc


def make_consts():
    mask = np.full((272, 256), MASKVAL, np.float32)
    rel = np.arange(271)[:, None]
    trel = np.arange(256)[None, :]
    band = (rel >= trel) & (rel <= trel + TP)
    mask[:271][band] = 0.0
    ident = np.eye(128, dtype=np.float32)
    onesp = np.zeros((E, 128, 2 * E), np.float32)
    for e in range(E):
        onesp[e, 0:64, 2 * e] = 1.0
        onesp[e, 64:128, 2 * e + 1] = 1.0
    ones4 = np.zeros((128, 2 * HPC), np.float32)
    ones4[:, 0::2] = 1.0
    zpad = np.zeros((128, TP), np.float32)
    return mask, ident, onesp, ones4, zpad


def make_in_maps(x, q_w, q_b, k_w, k_b, v_w, v_b, p_w):
    mask, ident, onesp, ones4, zpad = make_consts()
    in_maps = []
    for core in range(NCORES):
        b, g = core // HPC, core % HPC
        ch = slice(CH * g, CH * (g + 1))
        xTf = np.zeros((D, TPAD), np.float32)
        xTf[:, TP:] = x[b].T
        in_maps.append({
            "xT": np.ascontiguousarray(xTf),
            "qw": np.ascontiguousarray(q_w[ch].transpose(2, 1, 0)),
            "kw": np.ascontiguousarray(k_w[ch].transpose(2, 1, 0)),
            "vw": np.ascontiguousarray(v_w[ch].T),
            "pw": np.ascontiguousarray(p_w[:, ch].T),
            "qb": np.ascontiguousarray(q_b[ch][:, None]),
            "kb": np.ascontiguousarray(k_b[ch][:, None]),
            "vb": np.ascontiguousarray(v_b[ch][:, None]),
            "mask": mask, "ident": ident, "onesp": onesp,
            "ones4": ones4, "zpad": zpad,
            "vbrow": np.ascontiguousarray(v_b[ch][None, :]),
            "vzero": np.zeros((TP, CH), np.float32),
        })
    return in_maps


def assemble_output(results, p_b):
    out = np.zeros((B, T, D), np.float32)
    for core in range(NCORES):
        out[core // HPC] += results[core]["y"].T
    out += p_b[None, None, :]
    return out


def _run(inputs, trace=False):
    from concourse.bass_utils import run_bass_kernel_spmd
    nc = build_program()
    args = {k: np.asarray(v, np.float32) for k, v in inputs.items()}
    p_b = args.pop("p_b")
    in_maps = make_in_maps(**args)
    res = run_bass_kernel_spmd(nc, in_maps, list(range(NCORES)), trace=trace)
    out = assemble_output(res.results, p_b)
    return out, res


def kernel(**inputs):
    out, _ = _run(inputs)
    return out



# revision 45
# speedup vs baseline: 1.1082x; 1.1082x over previous
"""LogSparseAttention Trainium2 kernel (8-core SPMD), v2.

Sharding: 8 cores = 2 batches x 4 head-groups (4 heads = 256 channels each).
Each core: causal convs (q, k) for its 256 output channels over the full-D
input, v projection, local window-16 attention + 8 exponential-jump terms for
its 4 heads, then a partial output projection over its 256 channels.
Host sums the 8 partial [D, T] outputs (4 per batch) and adds p_b.

v2 over the 818us baseline:
  * all matmuls in bf16 (weights + activations; psum stays f32).  The f32r
    baseline was LDWEIGHTS-bound: every 512-row matmul paid a ~226ns 4-byte
    stationary load (640cyc = 280ns/matmul).  bf16 halves the load bytes so
    it hides under the 213ns moving stream.
  * k-convs run t4-block-sequential (one PSUM bank at a time) so local
    attention for a head-group starts while its own k-conv (later blocks)
    and the other group's convs still stream on the PE.
  * jump FMA for g0 overlaps q-conv of g1; g1's FMA is t4-blocked and
    pipelined with the output projection in the tail.

Layouts on chip (partition dim first):
  xT      [1024, 15+2048] bf16  x transposed, left-padded 15 zeros
  q_sb    2 x [128, 2048] bf16  conv out (+bias, x1/8 folded on host)
  k_sb    2 x [128, 15+2048] bf16  conv out (+bias), left-padded zeros
  v_dt    2 x [128, 2048] bf16  v (+bias), channel-partition (jump FMA)
  v_td    17 x [128, 4, 66] bf16  v (no bias) token-partition, +15-shifted,
                                per-head ones column (Z accumulator)
  z       2 x [128, 2048] f32   attention output accumulator [d, t]
  zr      2 x [128, 2048] bf16  z converted for the projection
"""
import sys

sys.path.insert(0, "/opt/trn_rl_repo")

import numpy as np
import ml_dtypes
import concourse.bass as bass
import concourse.bacc as bacc
import concourse.tile as tile
from concourse import mybir

f32 = mybir.dt.float32
bf16 = mybir.dt.bfloat16
f32r = mybir.dt.float32r
AL = mybir.AluOpType
AF = mybir.ActivationFunctionType

B, T, D = 2, 2048, 1024
H, W, E = 16, 16, 8
HD = D // H                  # 64
NCORES = 8
HPC = 4                      # heads per core
CH = HPC * HD                # 256 channels per core
NG = 2                       # head-pairs per core, 128 channels each
TP = W - 1                   # 15 (left pad)
TPAD = T + TP                # 2063
NT128 = T // 128             # 16
NT256 = T // 256             # 8
NT512 = T // 512             # 4
KT = D // 128                # 8 k-tiles over the input dim
SCALE = 1.0 / float(np.sqrt(HD))
MASKVAL = -200.0

_CACHE = {}


def build_program():
    if "nc" in _CACHE:
        return _CACHE["nc"]
    import contextlib
    import os as _os2
    # bisect aid: "conv" = v+conv only; "attn" = +local/jump; "all" = everything
    PH = _os2.environ.get("KERNEL_PHASES", "all")
    nc = bacc.Bacc()

    xT = nc.dram_tensor("xT", [D, TPAD], bf16, kind="ExternalInput")
    qw = nc.dram_tensor("qw", [W, D, CH], bf16, kind="ExternalInput")
    kw = nc.dram_tensor("kw", [W, D, CH], bf16, kind="ExternalInput")
    vw = nc.dram_tensor("vw", [D, CH], bf16, kind="ExternalInput")
    pw = nc.dram_tensor("pw", [CH, D], bf16, kind="ExternalInput")
    qb = nc.dram_tensor("qb", [CH, 1], f32, kind="ExternalInput")
    kb = nc.dram_tensor("kb", [CH, 1], f32, kind="ExternalInput")
    vb = nc.dram_tensor("vb", [CH, 1], f32, kind="ExternalInput")
    mask = nc.dram_tensor("mask", [272, 512], f32, kind="ExternalInput")
    ident = nc.dram_tensor("ident", [128, 128], f32r, kind="ExternalInput")
    onesp = nc.dram_tensor("onesp", [E, 128, 2 * E], bf16, kind="ExternalInput")
    ones4 = nc.dram_tensor("ones4", [128, 2 * HPC], f32r, kind="ExternalInput")
    zpad = nc.dram_tensor("zpad", [128, TP], f32r, kind="ExternalInput")
    vbrow = nc.dram_tensor("vbrow", [1, CH], f32, kind="ExternalInput")
    vzero = nc.dram_tensor("vzero", [TP, CH], f32r, kind="ExternalInput")
    y = nc.dram_tensor("y", [D, T], f32, kind="ExternalOutput")
    import os as _os
    DBG = bool(_os.environ.get("KERNEL_DEBUG"))
    if DBG:
        dbg_q = nc.dram_tensor("dbg_q", [CH, T], f32, kind="ExternalOutput")
        dbg_k = nc.dram_tensor("dbg_k", [CH, TPAD], f32, kind="ExternalOutput")
        dbg_v = nc.dram_tensor("dbg_v", [CH, T], bf16, kind="ExternalOutput")
        dbg_z = nc.dram_tensor("dbg_z", [CH, T], f32, kind="ExternalOutput")
    alpha_d = nc.dram_tensor("alpha_d0", [2 * E, T], bf16)
    zsum_d = [nc.dram_tensor(f"zsum_d{g}", [2 * E, 1], f32) for g in range(NG)]
    # sel[r, 128e + p] = 1 iff r == 2e + (p >= 64): PE-broadcast stationary
    seld = nc.dram_tensor("seld", [2 * E, E * 128], f32r, kind="ExternalInput")

    with tile.TileContext(nc) as tc:
        with contextlib.ExitStack() as ctx:
            consts = ctx.enter_context(tc.tile_pool(name="consts", bufs=1))
            main = ctx.enter_context(tc.tile_pool(name="main", bufs=1))

            # ---- constants ----
            m0 = consts.tile([128, 512], f32)
            m1 = consts.tile([128, 512], f32)
            m2 = consts.tile([TP, 512], f32)
            nc.sync.dma_start(m0[:], mask[0:128, :])
            nc.sync.dma_start(m1[:], mask[128:256, :])
            nc.sync.dma_start(m2[:], mask[256:271, :])
            id_sb = consts.tile([128, 128], f32r)
            nc.sync.dma_start(id_sb[:], ident[:])
            onesp_sb = consts.tile([128, E, 2 * E], bf16)
            nc.sync.dma_start(onesp_sb[:], onesp.rearrange("e p m -> p e m"))
            qb_sb = consts.tile([128, NG], f32)
            kb_sb = consts.tile([128, NG], f32)
            vb_sb = consts.tile([128, NG], f32)
            nc.sync.dma_start(qb_sb[:], qb.rearrange("(g p) o -> p (g o)", g=NG))
            nc.sync.dma_start(kb_sb[:], kb.rearrange("(g p) o -> p (g o)", g=NG))
            nc.sync.dma_start(vb_sb[:], vb.rearrange("(g p) o -> p (g o)", g=NG))
            pw_sb = [consts.tile([128, D], bf16, tag=f"pw{g}", name=f"pw_sb{g}") for g in range(NG)]
            for g in range(NG):
                nc.sync.dma_start(pw_sb[g][:], pw[128 * g:128 * (g + 1), :])
            vw_sb = [consts.tile([128, CH], bf16, tag=f"vw{i}", name=f"vw_sb{i}") for i in range(KT)]
            for i in range(KT):
                nc.sync.dma_start(vw_sb[i][:], vw[128 * i:128 * (i + 1), :])
            vbt = consts.tile([128, CH], f32)
            nc.sync.dma_start(vbt[:], vbrow[:].to_broadcast((128, CH)))
            sel_sb = consts.tile([2 * E, E * 128], f32r)
            nc.sync.dma_start(sel_sb[:], seld[:])

            # ---- persistent activations ----
            q_sb = [main.tile([128, T], f32r, tag=f"q{g}", name=f"q_sb{g}") for g in range(NG)]
            k_sb = [main.tile([128, TPAD], f32r, tag=f"k{g}", name=f"k_sb{g}") for g in range(NG)]
            v_dt = [main.tile([128, T], bf16, tag=f"vdt{g}", name=f"v_dt{g}") for g in range(NG)]
            v_td = [main.tile([128, HPC, HD + 2], f32r, tag=f"vtd{j}", name=f"v_td{j}")
                    for j in range(NT128 + 1)]
            z = [main.tile([128, T], f32, tag=f"z{g}", name=f"z{g}") for g in range(NG)]
            zr = [main.tile([128, T], bf16, tag=f"zr{g}", name=f"zr{g}") for g in range(NG)]
            # shared between the two head-groups (used strictly sequentially)
            erows = main.tile([2 * E, T], f32r, tag="er", name="erows")
            zsum = main.tile([2 * E, 1], f32, tag="zs", name="zsum")
            # per-partition 1/Z: zcol[g][p, e] = 1/Z[2e + (p >= 64)]
            zcol = [main.tile([128, E], f32, tag=f"zc{g}", name=f"zcol{g}")
                    for g in range(NG)]

            for g in range(NG):
                nc.sync.dma_start(k_sb[g][:, 0:TP], zpad[:])

            xpool = ctx.enter_context(tc.tile_pool(name="xw", bufs=1))
            xT_sb = [xpool.tile([128, TPAD], bf16, tag=f"x{i}", name=f"xT_sb{i}") for i in range(KT)]
            for i in range(KT):
                nc.sync.dma_start(xT_sb[i][:], xT[128 * i:128 * (i + 1), :])

            # ================= phase V: v projections =================
            with tc.tile_pool(name="psV", bufs=1, space="PSUM") as psV:
                # v in [t, d] layout (Form M=t), shifted +15, no bias
                for j in range(NT128 + 1):
                    mrow = 128 if j < NT128 else TP
                    pv = psV.tile([128, 512], f32, tag=f"vb{j % 2}", bufs=1, name=f"pv{j}")[:, 0:CH]
                    for i in range(KT):
                        nc.tensor.matmul(
                            pv[0:mrow, :],
                            xT_sb[i][:, 128 * j:128 * j + mrow],
                            vw_sb[i][:],
                            start=(i == 0), stop=(i == KT - 1),
                        )
                    nc.vector.tensor_tensor(
                        v_td[j][0:mrow, :, 0:HD],
                        pv[0:mrow, :].rearrange("p (h d) -> p h d", h=HPC),
                        vbt[0:mrow, :].rearrange("p (h d) -> p h d", h=HPC),
                        AL.add,
                    )
                    if j == 0:
                        # keys at t<0 are zero-padded AFTER bias in the reference
                        nc.sync.dma_start(
                            v_td[0][0:TP, :, 0:HD],
                            vzero.rearrange("p (h d) -> p h d", h=HPC))
                    nc.sync.dma_start(
                        v_td[j][:, :, HD:HD + 2],
                        ones4.rearrange("p (h o) -> p h o", o=2))

                # v in [o, t] layout (Form M=o), with bias
                for g in range(NG):
                    for t4 in range(NT512):
                        pv2 = psV.tile([128, 512], f32, tag=f"vb{2 + t4 % 2}", bufs=1, name=f"pv2_{g}_{t4}")
                        for i in range(KT):
                            nc.tensor.matmul(
                                pv2[:],
                                vw_sb[i][:, 128 * g:128 * (g + 1)],
                                xT_sb[i][:, TP + 512 * t4: TP + 512 * (t4 + 1)],
                                start=(i == 0), stop=(i == KT - 1),
                            )
                        nc.vector.tensor_scalar(
                            v_dt[g][:, 512 * t4:512 * (t4 + 1)], pv2[:],
                            vb_sb[:, g:g + 1], None, op0=AL.add,
                        )

            # ============ conv + attention, pipelined ============
            psA = ctx.enter_context(tc.tile_pool(name="psA", bufs=1, space="PSUM"))
            wpool = ctx.enter_context(tc.tile_pool(name="wstream", bufs=8))
            apool = ctx.enter_context(tc.tile_pool(name="attn", bufs=1))
            psC_cm = tc.tile_pool(name="psC", bufs=1, space="PSUM")
            psC = psC_cm.__enter__()

            def conv_block(wdram, bias_sb, dst, dst_off, g, b):
                """one conv token-block (512 tokens) for group g, one bank,
                accumulated over all (i, dt)."""
                pc = psC.tile([128, 512], f32, tag=f"cb{b % 2}", bufs=1,
                              name=f"pc{g}_{b}")
                for i in range(KT):
                    wt = wpool.tile([128, W, 128], bf16, tag="w")
                    nc.sync.dma_start(
                        wt[:], wdram[:, 128 * i:128 * (i + 1),
                                     128 * g:128 * (g + 1)].rearrange(
                                         "w p c -> p w c"))
                    for dt in range(W):
                        nc.tensor.matmul(
                            pc[:],
                            wt[:, dt, :],
                            xT_sb[i][:, 512 * b + dt:512 * b + dt + 512],
                            start=(i == 0 and dt == 0),
                            stop=(i == KT - 1 and dt == W - 1),
                        )
                nc.vector.tensor_scalar(
                    dst[:, dst_off + 512 * b:dst_off + 512 * (b + 1)], pc[:],
                    bias_sb[:, g:g + 1], None, op0=AL.add,
                )

            def local_attn_chunk(g, c):
                """local window attention for 256 queries [256c, 256c+256)."""
                exps = []
                for hh in range(2):
                    r0, r1 = 64 * hh, 64 * hh + 64
                    e0 = apool.tile([128, 256], f32r, tag="e0", bufs=3, name="e0")
                    e1 = apool.tile([128, 256], f32r, tag="e1", bufs=3, name="e1")
                    e2 = apool.tile([TP, 256], f32r, tag="e2", bufs=3, name="e2")
                    for (et, msk, s0, srows) in (
                        (e0, m0, 256 * c, 128),
                        (e1, m1, 256 * c + 128, 128),
                        (e2, m2, 256 * c + 256, TP),
                    ):
                        ps = psA.tile([128, 256], f32, tag="score", bufs=2, name="ps")
                        nc.tensor.matmul(
                            ps[0:srows, :],
                            k_sb[g][r0:r1, s0:s0 + srows],
                            q_sb[g][r0:r1, 256 * c:256 * (c + 1)],
                            start=True, stop=True,
                        )
                        nc.vector.tensor_tensor(
                            ps[0:srows, :], ps[0:srows, :],
                            msk[0:srows, 0:256], AL.add)
                        nc.scalar.activation(
                            et[0:srows, :], ps[0:srows, :], AF.Exp)
                    exps.append((e0, e1, e2))
                for sub in range(2):
                    jj = 2 * c + sub
                    stage = apool.tile([128, 128], f32r, tag="stage", bufs=2, name="stage")
                    for hh in range(2):
                        e0, e1, e2 = exps[hh]
                        if sub == 0:
                            lo, hi = e0[:, 0:128], e1[0:TP, 0:128]
                        else:
                            lo, hi = e1[:, 128:256], e2[0:TP, 128:256]
                        hl = 2 * g + hh
                        po = psA.tile([128, HD + 2], f32, tag="tp", bufs=2, name="po")
                        nc.tensor.matmul(
                            po[:], lo, v_td[jj][:, hl, :], start=True, stop=False)
                        nc.tensor.matmul(
                            po[:], hi, v_td[jj + 1][0:TP, hl, :],
                            start=False, stop=True)
                        rz = apool.tile([128, 1], f32, tag="rz", bufs=2, name="rz")
                        nc.vector.reciprocal(rz[:], po[:, HD:HD + 1])
                        nc.vector.tensor_scalar(
                            stage[:, 64 * hh:64 * hh + 64],
                            po[:, 0:HD], rz[:], None, op0=AL.mult,
                        )
                    pt = psA.tile([128, 128], f32r, tag="tp", bufs=2, name="pt")
                    nc.tensor.transpose(pt[:], stage[:], id_sb[:])
                    tcol = 128 * jj
                    nc.vector.tensor_copy(z[g][:, tcol:tcol + 128], pt[:].bitcast(f32))

            def jump_scores_block(g, t4):
                """jump scores for tokens [512 t4, 512 (t4+1)), accumulated over e."""
                pj = psA.tile([2 * E, 512], f32, tag="pj", bufs=1, name="pj")
                t0 = 512 * t4
                for e in range(E):
                    sh = 1 << e
                    tmp = apool.tile([128, 512], bf16, tag="jtmp", bufs=2, name="jtmp")
                    cut = min(512, T - sh - t0)
                    nc.vector.tensor_tensor(
                        tmp[:, 0:cut],
                        q_sb[g][:, t0:t0 + cut].bitcast(f32),
                        k_sb[g][:, TP + t0 + sh:TP + t0 + sh + cut].bitcast(f32),
                        AL.mult)
                    if cut < 512:
                        nc.vector.tensor_tensor(
                            tmp[:, cut:512],
                            q_sb[g][:, t0 + cut:t0 + 512].bitcast(f32),
                            k_sb[g][:, TP:TP + 512 - cut].bitcast(f32), AL.mult)
                    nc.tensor.matmul(
                        pj[:], onesp_sb[:, e, :], tmp[:],
                        start=(e == 0), stop=(e == E - 1),
                    )
                nc.scalar.activation(
                    erows[:, t0:t0 + 512], pj[:], AF.Exp)

            def jump_finalize(g):
                """1/Z per (e, head) into zcol[g]; g0 also stages unnormalized
                exp rows in DRAM for the DMA-broadcast FMA."""
                nc.vector.tensor_reduce(
                    zsum[:], erows[:].bitcast(f32), mybir.AxisListType.X, AL.add)
                nc.vector.reciprocal(zsum[:], zsum[:])
                nc.sync.dma_start(zsum_d[g][:], zsum[:])
                zv = zsum_d[g].rearrange("(e t) o -> t (e o)", t=2)
                nc.sync.dma_start(
                    zcol[g][0:64, :], zv[0:1, :].to_broadcast((64, E)))
                nc.sync.dma_start(
                    zcol[g][64:128, :], zv[1:2, :].to_broadcast((64, E)))
                if g == 0:
                    ebf = apool.tile([2 * E, T], bf16, tag="ebf", bufs=1, name="ebf")
                    nc.vector.tensor_copy(ebf[:], erows[:].bitcast(f32))
                    nc.sync.dma_start(alpha_d[:], ebf[:])

            def jump_fma_block(g, e, t4):
                """z[g][:, block t4] += (exp_e / Z_e) * v shifted, 512 cols.
                g0: broadcast unnormalized exp rows via DMA (PE is busy);
                g1 (tail): broadcast via a tiny PE matmul — no DRAM trip."""
                sh = 1 << e
                t0 = 512 * t4
                if g == 0:
                    bc = apool.tile([128, 512], bf16, tag="bc4", bufs=3, name="bc4")
                    nc.sync.dma_start(
                        bc[0:64, :],
                        alpha_d[2 * e:2 * e + 1, t0:t0 + 512].to_broadcast((64, 512)))
                    nc.sync.dma_start(
                        bc[64:128, :],
                        alpha_d[2 * e + 1:2 * e + 2, t0:t0 + 512].to_broadcast((64, 512)))
                else:
                    bc = psP.tile([128, 512], f32, tag="py", bufs=3, name="bcps")
                    nc.tensor.matmul(
                        bc[:], sel_sb[:, 128 * e:128 * (e + 1)],
                        erows[:, t0:t0 + 512],
                        start=True, stop=True,
                    )
                ft4 = apool.tile([128, 512], f32, tag="ft4", bufs=3, name="ft4")
                cut = min(512, T - sh - t0)
                # scalar_tensor_tensor only exists on DVE; gpsimd also
                # cannot read PSUM (g1's bc)
                eng = nc.vector
                eng.scalar_tensor_tensor(
                    ft4[:, 0:cut], bc[:, 0:cut], zcol[g][:, e:e + 1],
                    v_dt[g][:, t0 + sh:t0 + sh + cut], op0=AL.mult, op1=AL.mult)
                if cut < 512:
                    eng.scalar_tensor_tensor(
                        ft4[:, cut:512], bc[:, cut:512], zcol[g][:, e:e + 1],
                        v_dt[g][:, 0:512 - cut], op0=AL.mult, op1=AL.mult)
                nc.gpsimd.dma_start(
                    z[g][:, t0:t0 + 512], ft4[:], accum_op=AL.add)

            # ---- schedule ----
            def attn_units(g):
                loc = (lambda c: (lambda: local_attn_chunk(g, c))) if do_local \
                    else (lambda c: (lambda: None))
                jmp = (lambda t: (lambda: jump_scores_block(g, t))) if do_jump \
                    else (lambda t: (lambda: None))
                fin = (lambda: jump_finalize(g)) if do_jump else (lambda: None)
                return [
                    [loc(0), loc(1)],
                    [loc(2), loc(3), jmp(0)],
                    [loc(4), loc(5), jmp(1)],
                    [loc(6), loc(7), jmp(2), jmp(3), fin],
                ]

            do_local = PH in ("local", "attn", "fma", "all")
            do_jump = PH in ("jump", "attn", "fma", "all")
            do_tail = do_local or PH == "proj"
            do_fma = PH in ("fma", "all")
            if PH == "proj":
                for g in range(NG):
                    nc.vector.memset(z[g][:], 0.0)
            # q-g0, k-g0 (+ attn-g0 per k block)
            for b in range(NT512):
                conv_block(qw, qb_sb, q_sb[0], 0, 0, b)
            units0 = attn_units(0)
            for b in range(NT512):
                conv_block(kw, kb_sb, k_sb[0], TP, 0, b)
                for u in units0[b]:
                    u()
            # q-g1 (+ FMA-g0, t4-blocked: 8 units per block)
            for b in range(NT512):
                conv_block(qw, qb_sb, q_sb[1], 0, 1, b)
                if do_fma:
                    for e in range(E):
                        jump_fma_block(0, e, b)
            if do_tail:
                nc.vector.tensor_copy(zr[0][:], z[0][:])
            # k-g1 (+ attn-g1 per block)
            units1 = attn_units(1)
            for b in range(NT512):
                conv_block(kw, kb_sb, k_sb[1], TP, 1, b)
                for u in units1[b]:
                    u()
            psC_cm.__exit__(None, None, None)

            if DBG:
                for g in range(NG):
                    nc.sync.dma_start(dbg_q[128 * g:128 * (g + 1), :], q_sb[g][:])
                    nc.sync.dma_start(dbg_k[128 * g:128 * (g + 1), :], k_sb[g][:])
                    nc.sync.dma_start(dbg_v[128 * g:128 * (g + 1), :], v_dt[g][:])

            # ---- tail: FMA-g1 t4-blocked, pipelined with projection ----
            with tc.tile_pool(name="psP", bufs=1, space="PSUM") as psP:
                for t4 in range(NT512 if do_tail else 0):
                    if do_fma:
                        for e in range(E):
                            jump_fma_block(1, e, t4)
                    t0 = 512 * t4
                    nc.vector.tensor_copy(
                        zr[1][:, t0:t0 + 512], z[1][:, t0:t0 + 512])
                    for o8 in range(D // 128):
                        py = psP.tile([128, 512], f32, tag="py", bufs=3, name="py")
                        for g in range(NG):
                            nc.tensor.matmul(
                                py[:],
                                pw_sb[g][:, 128 * o8:128 * (o8 + 1)],
                                zr[g][:, t0:t0 + 512],
                                start=(g == 0), stop=(g == NG - 1),
                            )
                        ysb = apool.tile([128, 512], f32, tag="ysb", bufs=3, name="ysb")
                        if o8 % 2 == 0:
                            nc.scalar.copy(ysb[:], py[:])
                        else:
                            nc.vector.tensor_copy(ysb[:], py[:])
                        nc.sync.dma_start(
                            y[128 * o8:128 * (o8 + 1), t0:t0 + 512], ysb[:])
                if DBG:
                    for g in range(NG):
                        nc.sync.dma_start(dbg_z[128 * g:128 * (g + 1), :], z[g][:])

    nc.compile()
    _CACHE["nc"] = nc
    return nc


def make_consts():
    mask = np.full((272, 256), MASKVAL, np.float32)
    rel = np.arange(271)[:, None]
    trel = np.arange(256)[None, :]
    band = (rel >= trel) & (rel <= trel + TP)
    mask[:271][band] = 0.0
    mask2 = np.tile(mask, (1, 2))  # duplicated for the two heads
    ident = np.eye(128, dtype=np.float32)
    onesp = np.zeros((E, 128, 2 * E), ml_dtypes.bfloat16)
    for e in range(E):
        onesp[e, 0:64, 2 * e] = 1.0
        onesp[e, 64:128, 2 * e + 1] = 1.0
    ones4 = np.zeros((128, 2 * HPC), np.float32)
    ones4[:, 0::2] = 1.0
    zpad = np.zeros((128, TP), np.float32)
    seld = np.zeros((2 * E, E * 128), np.float32)
    for e in range(E):
        seld[2 * e, 128 * e:128 * e + 64] = 1.0
        seld[2 * e + 1, 128 * e + 64:128 * (e + 1)] = 1.0
    return mask2, ident, onesp, ones4, zpad, seld


def make_in_maps(x, q_w, q_b, k_w, k_b, v_w, v_b, p_w):
    mask, ident, onesp, ones4, zpad, seld = make_consts()
    bf = ml_dtypes.bfloat16
    in_maps = []
    for core in range(NCORES):
        b, g = core // HPC, core % HPC
        ch = slice(CH * g, CH * (g + 1))
        xTf = np.zeros((D, TPAD), np.float32)
        xTf[:, TP:] = x[b].T
        in_maps.append({
            "xT": np.ascontiguousarray(xTf).astype(bf),
            # q path pre-scaled by 1/sqrt(HD)
            "qw": np.ascontiguousarray(q_w[ch].transpose(2, 1, 0) * SCALE).astype(bf),
            "kw": np.ascontiguousarray(k_w[ch].transpose(2, 1, 0)).astype(bf),
            "vw": np.ascontiguousarray(v_w[ch].T).astype(bf),
            "pw": np.ascontiguousarray(p_w[:, ch].T).astype(bf),
            "qb": np.ascontiguousarray((q_b[ch] * SCALE)[:, None]),
            "kb": np.ascontiguousarray(k_b[ch][:, None]),
            "vb": np.ascontiguousarray(v_b[ch][:, None]),
            "mask": mask, "ident": ident, "onesp": onesp,
            "ones4": ones4, "zpad": zpad, "seld": seld,
            "vbrow": np.ascontiguousarray(v_b[ch][None, :]),
            "vzero": np.zeros((TP, CH), np.float32),
        })
    return in_maps


def assemble_output(results, p_b):
    out = np.zeros((B, T, D), np.float32)
    for core in range(NCORES):
        out[core // HPC] += results[core]["y"].T
    out += p_b[None, None, :]
    return out


def _run(inputs, trace=False):
    from concourse.bass_utils import run_bass_kernel_spmd
    nc = build_program()
    args = {k: np.asarray(v, np.float32) for k, v in inputs.items()}
    p_b = args.pop("p_b")
    in_maps = make_in_maps(**args)
    res = run_bass_kernel_spmd(nc, in_maps, list(range(NCORES)), trace=trace)
    out = assemble_output(res.results, p_b)
    return out, res


def kernel(**inputs):
    out, _ = _run(inputs)
    return out


# revision 52
# speedup vs baseline: 1.2715x; 1.1473x over previous
"""LogSparseAttention Trainium2 kernel (8-core SPMD), v2.

Sharding: 8 cores = 2 batches x 4 head-groups (4 heads = 256 channels each).
Each core: causal convs (q, k) for its 256 output channels over the full-D
input, v projection, local window-16 attention + 8 exponential-jump terms for
its 4 heads, then a partial output projection over its 256 channels.
Host sums the 8 partial [D, T] outputs (4 per batch) and adds p_b.

v2 over the 818us baseline:
  * all matmuls in bf16 (weights + activations; psum stays f32).  The f32r
    baseline was LDWEIGHTS-bound: every 512-row matmul paid a ~226ns 4-byte
    stationary load (640cyc = 280ns/matmul).  bf16 halves the load bytes so
    it hides under the 213ns moving stream.
  * k-convs run t4-block-sequential (one PSUM bank at a time) so local
    attention for a head-group starts while its own k-conv (later blocks)
    and the other group's convs still stream on the PE.
  * jump FMA for g0 overlaps q-conv of g1; g1's FMA is t4-blocked and
    pipelined with the output projection in the tail.

Layouts on chip (partition dim first):
  xT      [1024, 15+2048] bf16  x transposed, left-padded 15 zeros
  q_sb    2 x [128, 2048] bf16  conv out (+bias, x1/8 folded on host)
  k_sb    2 x [128, 15+2048] bf16  conv out (+bias), left-padded zeros
  v_dt    2 x [128, 2048] bf16  v (+bias), channel-partition (jump FMA)
  v_td    17 x [128, 4, 66] bf16  v (no bias) token-partition, +15-shifted,
                                per-head ones column (Z accumulator)
  z       2 x [128, 2048] f32   attention output accumulator [d, t]
  zr      2 x [128, 2048] bf16  z converted for the projection
"""
import sys

sys.path.insert(0, "/opt/trn_rl_repo")

import numpy as np
import ml_dtypes
import concourse.bass as bass
import concourse.bacc as bacc
import concourse.tile as tile
from concourse import mybir

f32 = mybir.dt.float32
bf16 = mybir.dt.bfloat16
f32r = mybir.dt.float32r
AL = mybir.AluOpType
AF = mybir.ActivationFunctionType

B, T, D = 2, 2048, 1024
H, W, E = 16, 16, 8
HD = D // H                  # 64
NCORES = 8
HPC = 4                      # heads per core
CH = HPC * HD                # 256 channels per core
NG = 2                       # head-pairs per core, 128 channels each
TP = W - 1                   # 15 (left pad)
TPAD = T + TP                # 2063
NT128 = T // 128             # 16
NT256 = T // 256             # 8
NT512 = T // 512             # 4
KT = D // 128                # 8 k-tiles over the input dim
SCALE = 1.0 / float(np.sqrt(HD))
MASKVAL = -200.0

_CACHE = {}


def build_program():
    if "nc" in _CACHE:
        return _CACHE["nc"]
    import contextlib
    import os as _os2
    # bisect aid: "conv" = v+conv only; "attn" = +local/jump; "all" = everything
    PH = _os2.environ.get("KERNEL_PHASES", "all")
    nc = bacc.Bacc()

    xT = nc.dram_tensor("xT", [D, TPAD], bf16, kind="ExternalInput")
    qw = nc.dram_tensor("qw", [W, D, CH], bf16, kind="ExternalInput")
    kw = nc.dram_tensor("kw", [W, D, CH], bf16, kind="ExternalInput")
    vw = nc.dram_tensor("vw", [D, CH], bf16, kind="ExternalInput")
    pw = nc.dram_tensor("pw", [CH, D], bf16, kind="ExternalInput")
    qb = nc.dram_tensor("qb", [CH, 1], f32, kind="ExternalInput")
    kb = nc.dram_tensor("kb", [CH, 1], f32, kind="ExternalInput")
    vb = nc.dram_tensor("vb", [CH, 1], f32, kind="ExternalInput")
    mask = nc.dram_tensor("mask", [272, 512], f32, kind="ExternalInput")
    ident = nc.dram_tensor("ident", [128, 128], f32r, kind="ExternalInput")
    onesp = nc.dram_tensor("onesp", [E, 128, 2 * E], bf16, kind="ExternalInput")
    ones4 = nc.dram_tensor("ones4", [128, 2 * HPC], f32r, kind="ExternalInput")
    zpad = nc.dram_tensor("zpad", [128, TP], f32r, kind="ExternalInput")
    vbrow = nc.dram_tensor("vbrow", [1, CH], f32, kind="ExternalInput")
    vzero = nc.dram_tensor("vzero", [TP, CH], f32r, kind="ExternalInput")
    y = nc.dram_tensor("y", [D, T], f32, kind="ExternalOutput")
    import os as _os
    DBG = bool(_os.environ.get("KERNEL_DEBUG"))
    if DBG:
        dbg_q = nc.dram_tensor("dbg_q", [CH, T], f32, kind="ExternalOutput")
        dbg_k = nc.dram_tensor("dbg_k", [CH, TPAD], f32, kind="ExternalOutput")
        dbg_v = nc.dram_tensor("dbg_v", [CH, T], bf16, kind="ExternalOutput")
        dbg_z = nc.dram_tensor("dbg_z", [CH, T], f32, kind="ExternalOutput")
    alpha_d = nc.dram_tensor("alpha_d0", [2 * E, T], bf16)
    zsum_d = nc.dram_tensor("zsum_d1", [2 * E, 1], f32)
    # sel[r, 128e + p] = 1 iff r == 2e + (p >= 64): PE-broadcast stationary
    seld = nc.dram_tensor("seld", [2 * E, E * 128], f32r, kind="ExternalInput")

    with tile.TileContext(nc) as tc:
        with contextlib.ExitStack() as ctx:
            consts = ctx.enter_context(tc.tile_pool(name="consts", bufs=1))
            main = ctx.enter_context(tc.tile_pool(name="main", bufs=1))

            # ---- constants ----
            m0 = consts.tile([128, 512], f32)
            m1 = consts.tile([128, 512], f32)
            m2 = consts.tile([TP, 512], f32)
            nc.sync.dma_start(m0[:], mask[0:128, :])
            nc.sync.dma_start(m1[:], mask[128:256, :])
            nc.sync.dma_start(m2[:], mask[256:271, :])
            id_sb = consts.tile([128, 128], f32r)
            nc.sync.dma_start(id_sb[:], ident[:])
            onesp_sb = consts.tile([128, E, 2 * E], bf16)
            nc.sync.dma_start(onesp_sb[:], onesp.rearrange("e p m -> p e m"))
            qb_sb = consts.tile([128, NG], f32)
            kb_sb = consts.tile([128, NG], f32)
            vb_sb = consts.tile([128, NG], f32)
            nc.sync.dma_start(qb_sb[:], qb.rearrange("(g p) o -> p (g o)", g=NG))
            nc.sync.dma_start(kb_sb[:], kb.rearrange("(g p) o -> p (g o)", g=NG))
            nc.sync.dma_start(vb_sb[:], vb.rearrange("(g p) o -> p (g o)", g=NG))
            pw_sb = [consts.tile([128, D], bf16, tag=f"pw{g}", name=f"pw_sb{g}") for g in range(NG)]
            for g in range(NG):
                nc.sync.dma_start(pw_sb[g][:], pw[128 * g:128 * (g + 1), :])
            vw_sb = [consts.tile([128, CH], bf16, tag=f"vw{i}", name=f"vw_sb{i}") for i in range(KT)]
            for i in range(KT):
                nc.sync.dma_start(vw_sb[i][:], vw[128 * i:128 * (i + 1), :])
            vbt = consts.tile([128, CH], f32)
            nc.sync.dma_start(vbt[:], vbrow[:].to_broadcast((128, CH)))
            sel_sb = consts.tile([2 * E, E * 128], f32r)
            nc.sync.dma_start(sel_sb[:], seld[:])

            # ---- persistent activations ----
            q_sb = [main.tile([128, T], f32r, tag=f"q{g}", name=f"q_sb{g}") for g in range(NG)]
            k_sb = [main.tile([128, TPAD], f32r, tag=f"k{g}", name=f"k_sb{g}") for g in range(NG)]
            v_dt = [main.tile([128, T], bf16, tag=f"vdt{g}", name=f"v_dt{g}") for g in range(NG)]
            v_td = [main.tile([128, HPC, HD + 2], f32r, tag=f"vtd{j}", name=f"v_td{j}")
                    for j in range(NT128 + 1)]
            z = [main.tile([128, T], f32, tag=f"z{g}", name=f"z{g}") for g in range(NG)]
            zr = [main.tile([128, T], bf16, tag=f"zr{g}", name=f"zr{g}") for g in range(NG)]
            # shared between the two head-groups (used strictly sequentially)
            erows = main.tile([2 * E, T], f32r, tag="er", name="erows")
            zsum = main.tile([2 * E, 1], f32, tag="zs", name="zsum")
            # per-partition 1/Z for g1's tail FMA: zcol[p, e] = 1/Z[2e + (p>=64)]
            zcol = main.tile([128, E], f32, tag="zc", name="zcol")

            for g in range(NG):
                nc.sync.dma_start(k_sb[g][:, 0:TP], zpad[:])

            xpool = ctx.enter_context(tc.tile_pool(name="xw", bufs=1))
            xT_sb = [xpool.tile([128, TPAD], bf16, tag=f"x{i}", name=f"xT_sb{i}") for i in range(KT)]
            for i in range(KT):
                nc.sync.dma_start(xT_sb[i][:], xT[128 * i:128 * (i + 1), :])

            # ================= phase V: v projections =================
            with tc.tile_pool(name="psV", bufs=1, space="PSUM") as psV:
                # v in [t, d] layout (Form M=t), shifted +15, no bias
                for j in range(NT128 + 1):
                    mrow = 128 if j < NT128 else TP
                    pv = psV.tile([128, 512], f32, tag=f"vb{j % 2}", bufs=1, name=f"pv{j}")[:, 0:CH]
                    for i in range(KT):
                        nc.tensor.matmul(
                            pv[0:mrow, :],
                            xT_sb[i][:, 128 * j:128 * j + mrow],
                            vw_sb[i][:],
                            start=(i == 0), stop=(i == KT - 1),
                        )
                    nc.vector.tensor_tensor(
                        v_td[j][0:mrow, :, 0:HD],
                        pv[0:mrow, :].rearrange("p (h d) -> p h d", h=HPC),
                        vbt[0:mrow, :].rearrange("p (h d) -> p h d", h=HPC),
                        AL.add,
                    )
                    if j == 0:
                        # keys at t<0 are zero-padded AFTER bias in the reference
                        nc.sync.dma_start(
                            v_td[0][0:TP, :, 0:HD],
                            vzero.rearrange("p (h d) -> p h d", h=HPC))
                    nc.sync.dma_start(
                        v_td[j][:, :, HD:HD + 2],
                        ones4.rearrange("p (h o) -> p h o", o=2))

                # v in [o, t] layout (Form M=o), with bias
                for g in range(NG):
                    for t4 in range(NT512):
                        pv2 = psV.tile([128, 512], f32, tag=f"vb{2 + t4 % 2}", bufs=1, name=f"pv2_{g}_{t4}")
                        for i in range(KT):
                            nc.tensor.matmul(
                                pv2[:],
                                vw_sb[i][:, 128 * g:128 * (g + 1)],
                                xT_sb[i][:, TP + 512 * t4: TP + 512 * (t4 + 1)],
                                start=(i == 0), stop=(i == KT - 1),
                            )
                        nc.vector.tensor_scalar(
                            v_dt[g][:, 512 * t4:512 * (t4 + 1)], pv2[:],
                            vb_sb[:, g:g + 1], None, op0=AL.add,
                        )

            # ============ conv + attention, pipelined ============
            psA = ctx.enter_context(tc.tile_pool(name="psA", bufs=1, space="PSUM"))
            wpool = ctx.enter_context(tc.tile_pool(name="wstream", bufs=8))
            apool = ctx.enter_context(tc.tile_pool(name="attn", bufs=1))
            psC_cm = tc.tile_pool(name="psC", bufs=1, space="PSUM")
            psC = psC_cm.__enter__()

            def conv_block(wdram, bias_sb, dst, dst_off, g, b):
                """one conv token-block (512 tokens) for group g, one bank,
                accumulated over all (i, dt)."""
                pc = psC.tile([128, 512], f32, tag=f"cb{b % 2}", bufs=1,
                              name=f"pc{g}_{b}")
                for i in range(KT):
                    wt = wpool.tile([128, W, 128], bf16, tag="w")
                    nc.sync.dma_start(
                        wt[:], wdram[:, 128 * i:128 * (i + 1),
                                     128 * g:128 * (g + 1)].rearrange(
                                         "w p c -> p w c"))
                    for dt in range(W):
                        nc.tensor.matmul(
                            pc[:],
                            wt[:, dt, :],
                            xT_sb[i][:, 512 * b + dt:512 * b + dt + 512],
                            start=(i == 0 and dt == 0),
                            stop=(i == KT - 1 and dt == W - 1),
                        )
                # bias-add on the (otherwise idle) Activation engine so DVE
                # backlog can't delay freeing the conv PSUM bank
                nc.scalar.activation(
                    dst[:, dst_off + 512 * b:dst_off + 512 * (b + 1)], pc[:],
                    AF.Identity, bias=bias_sb[:, g:g + 1],
                )

            def local_attn_chunk(g, c):
                """local window attention for 256 queries [256c, 256c+256)."""
                exps = []
                for hh in range(2):
                    r0, r1 = 64 * hh, 64 * hh + 64
                    e0 = apool.tile([128, 256], f32r, tag="e0", bufs=3, name="e0")
                    e1 = apool.tile([128, 256], f32r, tag="e1", bufs=3, name="e1")
                    e2 = apool.tile([TP, 256], f32r, tag="e2", bufs=3, name="e2")
                    for (et, msk, s0, srows) in (
                        (e0, m0, 256 * c, 128),
                        (e1, m1, 256 * c + 128, 128),
                        (e2, m2, 256 * c + 256, TP),
                    ):
                        ps = psA.tile([128, 256], f32, tag="score", bufs=2, name="ps")
                        nc.tensor.matmul(
                            ps[0:srows, :],
                            k_sb[g][r0:r1, s0:s0 + srows],
                            q_sb[g][r0:r1, 256 * c:256 * (c + 1)],
                            start=True, stop=True,
                        )
                        nc.vector.tensor_tensor(
                            ps[0:srows, :], ps[0:srows, :],
                            msk[0:srows, 0:256], AL.add)
                        nc.scalar.activation(
                            et[0:srows, :], ps[0:srows, :], AF.Exp)
                    exps.append((e0, e1, e2))
                for sub in range(2):
                    jj = 2 * c + sub
                    stage = apool.tile([128, 128], f32r, tag="stage", bufs=2, name="stage")
                    for hh in range(2):
                        e0, e1, e2 = exps[hh]
                        if sub == 0:
                            lo, hi = e0[:, 0:128], e1[0:TP, 0:128]
                        else:
                            lo, hi = e1[:, 128:256], e2[0:TP, 128:256]
                        hl = 2 * g + hh
                        po = psA.tile([128, HD + 2], f32, tag="tp", bufs=2, name="po")
                        nc.tensor.matmul(
                            po[:], lo, v_td[jj][:, hl, :], start=True, stop=False)
                        nc.tensor.matmul(
                            po[:], hi, v_td[jj + 1][0:TP, hl, :],
                            start=False, stop=True)
                        rz = apool.tile([128, 1], f32, tag="rz", bufs=2, name="rz")
                        nc.vector.reciprocal(rz[:], po[:, HD:HD + 1])
                        nc.vector.tensor_scalar(
                            stage[:, 64 * hh:64 * hh + 64],
                            po[:, 0:HD], rz[:], None, op0=AL.mult,
                        )
                    pt = psA.tile([128, 128], f32r, tag="tp", bufs=2, name="pt")
                    nc.tensor.transpose(pt[:], stage[:], id_sb[:])
                    tcol = 128 * jj
                    nc.vector.tensor_copy(z[g][:, tcol:tcol + 128], pt[:].bitcast(f32))

            def jump_scores_block(g, t4):
                """jump scores for tokens [512 t4, 512 (t4+1)), accumulated over e."""
                pj = psA.tile([2 * E, 512], f32, tag="pj", bufs=1, name="pj")
                t0 = 512 * t4
                for e in range(E):
                    sh = 1 << e
                    tmp = apool.tile([128, 512], bf16, tag="jtmp", bufs=2, name="jtmp")
                    cut = min(512, T - sh - t0)
                    nc.vector.tensor_tensor(
                        tmp[:, 0:cut],
                        q_sb[g][:, t0:t0 + cut].bitcast(f32),
                        k_sb[g][:, TP + t0 + sh:TP + t0 + sh + cut].bitcast(f32),
                        AL.mult)
                    if cut < 512:
                        nc.vector.tensor_tensor(
                            tmp[:, cut:512],
                            q_sb[g][:, t0 + cut:t0 + 512].bitcast(f32),
                            k_sb[g][:, TP:TP + 512 - cut].bitcast(f32), AL.mult)
                    nc.tensor.matmul(
                        pj[:], onesp_sb[:, e, :], tmp[:],
                        start=(e == 0), stop=(e == E - 1),
                    )
                nc.scalar.activation(
                    erows[:, t0:t0 + 512], pj[:], AF.Exp)

            def jump_finalize(g):
                """normalize: g0 stages alpha rows in DRAM for DMA-broadcast;
                g1 keeps unnormalized erows + a per-partition 1/Z column."""
                nc.vector.tensor_reduce(
                    zsum[:], erows[:].bitcast(f32), mybir.AxisListType.X, AL.add)
                nc.vector.reciprocal(zsum[:], zsum[:])
                if g == 0:
                    arows = apool.tile([2 * E, T], bf16, tag="ar", bufs=1,
                                       name="arows")
                    nc.vector.tensor_scalar(
                        arows[:], erows[:].bitcast(f32), zsum[:], None,
                        op0=AL.mult)
                    nc.sync.dma_start(alpha_d[:], arows[:])
                else:
                    nc.sync.dma_start(zsum_d[:], zsum[:])
                    zv = zsum_d.rearrange("(e t) o -> t (e o)", t=2)
                    nc.sync.dma_start(
                        zcol[0:64, :], zv[0:1, :].to_broadcast((64, E)))
                    nc.sync.dma_start(
                        zcol[64:128, :], zv[1:2, :].to_broadcast((64, E)))

            def jump_fma_block(g, e, t4):
                """g0: z0[:, block] += alpha_e*v via gpsimd mult + accum-DMA
                (hidden under q-g1's conv; DVE stays free for attention)."""
                sh = 1 << e
                t0 = 512 * t4
                bc = apool.tile([128, 512], bf16, tag="bc4", bufs=3, name="bc4")
                nc.sync.dma_start(
                    bc[0:64, :],
                    alpha_d[2 * e:2 * e + 1, t0:t0 + 512].to_broadcast((64, 512)))
                nc.sync.dma_start(
                    bc[64:128, :],
                    alpha_d[2 * e + 1:2 * e + 2, t0:t0 + 512].to_broadcast((64, 512)))
                ft4 = apool.tile([128, 512], f32, tag="ft4", bufs=2, name="ft4")
                cut = min(512, T - sh - t0)
                nc.gpsimd.tensor_tensor(
                    ft4[:, 0:cut], bc[:, 0:cut],
                    v_dt[g][:, t0 + sh:t0 + sh + cut], AL.mult)
                if cut < 512:
                    nc.gpsimd.tensor_tensor(
                        ft4[:, cut:512], bc[:, cut:512],
                        v_dt[g][:, 0:512 - cut], AL.mult)
                nc.gpsimd.dma_start(
                    z[g][:, t0:t0 + 512], ft4[:], accum_op=AL.add)

            def jump_fma_tail_block(t4):
                """g1: PE-broadcast unnormalized exp rows, DVE stt with 1/Z,
                accumulate in SBUF (two engine chains), fuse into zr1 —
                no serial accumulate-DMAs."""
                t0 = 512 * t4
                accA = apool.tile([128, 512], f32, tag="accA", bufs=1, name="accA")
                accB = apool.tile([128, 512], f32, tag="accB", bufs=1, name="accB")
                for e in range(E):
                    sh = 1 << e
                    bc = psP.tile([128, 512], f32, tag="py", bufs=3, name="bcps")
                    nc.tensor.matmul(
                        bc[:], sel_sb[:, 128 * e:128 * (e + 1)],
                        erows[:, t0:t0 + 512],
                        start=True, stop=True,
                    )
                    cut = min(512, T - sh - t0)
                    dst = accA if e == 0 else accB if e == 1 else \
                        apool.tile([128, 512], f32, tag="ft4", bufs=2, name="ft4")
                    nc.vector.scalar_tensor_tensor(
                        dst[:, 0:cut], bc[:, 0:cut], zcol[:, e:e + 1],
                        v_dt[1][:, t0 + sh:t0 + sh + cut],
                        op0=AL.mult, op1=AL.mult)
                    if cut < 512:
                        nc.vector.scalar_tensor_tensor(
                            dst[:, cut:512], bc[:, cut:512], zcol[:, e:e + 1],
                            v_dt[1][:, 0:512 - cut], op0=AL.mult, op1=AL.mult)
                    if e >= 2:
                        acc = accA if e % 2 == 0 else accB
                        eng = nc.gpsimd if e % 2 == 0 else nc.vector
                        eng.tensor_tensor(acc[:], acc[:], dst[:], AL.add)
                nc.vector.tensor_tensor(accA[:], accA[:], accB[:], AL.add)
                # fused: zr1 = bf16(z1_local + jump)   (z[1] itself stays local)
                nc.vector.tensor_tensor(
                    zr[1][:, t0:t0 + 512], z[1][:, t0:t0 + 512], accA[:], AL.add)

            # ---- schedule ----
            def attn_units(g):
                loc = (lambda c: (lambda: local_attn_chunk(g, c))) if do_local \
                    else (lambda c: (lambda: None))
                jmp = (lambda t: (lambda: jump_scores_block(g, t))) if do_jump \
                    else (lambda t: (lambda: None))
                fin = (lambda: jump_finalize(g)) if do_jump else (lambda: None)
                return [
                    [loc(0), loc(1)],
                    [loc(2), loc(3), jmp(0)],
                    [loc(4), loc(5), jmp(1)],
                    [loc(6), loc(7), jmp(2), jmp(3), fin],
                ]

            do_local = PH in ("local", "attn", "fma", "all")
            do_jump = PH in ("jump", "attn", "fma", "all")
            do_tail = do_local or PH == "proj"
            do_fma = PH in ("fma", "all")
            if PH == "proj":
                for g in range(NG):
                    nc.vector.memset(z[g][:], 0.0)
            # q-g0, k-g0 (+ attn-g0 per k block)
            for b in range(NT512):
                conv_block(qw, qb_sb, q_sb[0], 0, 0, b)
            units0 = attn_units(0)
            for b in range(NT512):
                conv_block(kw, kb_sb, k_sb[0], TP, 0, b)
                for u in units0[b]:
                    u()
            # q-g1 (+ FMA-g0, t4-blocked: 8 units per block)
            for b in range(NT512):
                conv_block(qw, qb_sb, q_sb[1], 0, 1, b)
                if do_fma:
                    for e in range(E):
                        jump_fma_block(0, e, b)
            if do_tail:
                nc.vector.tensor_copy(zr[0][:], z[0][:])
            # k-g1 (+ attn-g1 per block)
            units1 = attn_units(1)
            for b in range(NT512):
                conv_block(kw, kb_sb, k_sb[1], TP, 1, b)
                for u in units1[b]:
                    u()
            psC_cm.__exit__(None, None, None)

            if DBG:
                for g in range(NG):
                    nc.sync.dma_start(dbg_q[128 * g:128 * (g + 1), :], q_sb[g][:])
                    nc.sync.dma_start(dbg_k[128 * g:128 * (g + 1), :], k_sb[g][:])
                    nc.sync.dma_start(dbg_v[128 * g:128 * (g + 1), :], v_dt[g][:])

            # ---- tail: FMA-g1 t4-blocked, pipelined with projection ----
            with tc.tile_pool(name="psP", bufs=1, space="PSUM") as psP:
                for t4 in range(NT512 if do_tail else 0):
                    t0 = 512 * t4
                    if do_fma:
                        jump_fma_tail_block(t4)
                    else:
                        nc.vector.tensor_copy(
                            zr[1][:, t0:t0 + 512], z[1][:, t0:t0 + 512])
                    for o8 in range(D // 128):
                        py = psP.tile([128, 512], f32, tag="py", bufs=3, name="py")
                        for g in range(NG):
                            nc.tensor.matmul(
                                py[:],
                                pw_sb[g][:, 128 * o8:128 * (o8 + 1)],
                                zr[g][:, t0:t0 + 512],
                                start=(g == 0), stop=(g == NG - 1),
                            )
                        ysb = apool.tile([128, 512], f32, tag="ysb", bufs=3, name="ysb")
                        if o8 % 2 == 0:
                            nc.scalar.copy(ysb[:], py[:])
                        else:
                            nc.vector.tensor_copy(ysb[:], py[:])
                        nc.sync.dma_start(
                            y[128 * o8:128 * (o8 + 1), t0:t0 + 512], ysb[:])
                if DBG:
                    for g in range(NG):
                        nc.sync.dma_start(dbg_z[128 * g:128 * (g + 1), :], z[g][:])

    nc.compile()
    _CACHE["nc"] = nc
    return nc


def make_consts():
    mask = np.full((272, 256), MASKVAL, np.float32)
    rel = np.arange(271)[:, None]
    trel = np.arange(256)[None, :]
    band = (rel >= trel) & (rel <= trel + TP)
    mask[:271][band] = 0.0
    mask2 = np.tile(mask, (1, 2))  # duplicated for the two heads
    ident = np.eye(128, dtype=np.float32)
    onesp = np.zeros((E, 128, 2 * E), ml_dtypes.bfloat16)
    for e in range(E):
        onesp[e, 0:64, 2 * e] = 1.0
        onesp[e, 64:128, 2 * e + 1] = 1.0
    ones4 = np.zeros((128, 2 * HPC), np.float32)
    ones4[:, 0::2] = 1.0
    zpad = np.zeros((128, TP), np.float32)
    seld = np.zeros((2 * E, E * 128), np.float32)
    for e in range(E):
        seld[2 * e, 128 * e:128 * e + 64] = 1.0
        seld[2 * e + 1, 128 * e + 64:128 * (e + 1)] = 1.0
    return mask2, ident, onesp, ones4, zpad, seld


def make_in_maps(x, q_w, q_b, k_w, k_b, v_w, v_b, p_w):
    mask, ident, onesp, ones4, zpad, seld = make_consts()
    bf = ml_dtypes.bfloat16
    in_maps = []
    for core in range(NCORES):
        b, g = core // HPC, core % HPC
        ch = slice(CH * g, CH * (g + 1))
        xTf = np.zeros((D, TPAD), np.float32)
        xTf[:, TP:] = x[b].T
        in_maps.append({
            "xT": np.ascontiguousarray(xTf).astype(bf),
            # q path pre-scaled by 1/sqrt(HD)
            "qw": np.ascontiguousarray(q_w[ch].transpose(2, 1, 0) * SCALE).astype(bf),
            "kw": np.ascontiguousarray(k_w[ch].transpose(2, 1, 0)).astype(bf),
            "vw": np.ascontiguousarray(v_w[ch].T).astype(bf),
            "pw": np.ascontiguousarray(p_w[:, ch].T).astype(bf),
            "qb": np.ascontiguousarray((q_b[ch] * SCALE)[:, None]),
            "kb": np.ascontiguousarray(k_b[ch][:, None]),
            "vb": np.ascontiguousarray(v_b[ch][:, None]),
            "mask": mask, "ident": ident, "onesp": onesp,
            "ones4": ones4, "zpad": zpad, "seld": seld,
            "vbrow": np.ascontiguousarray(v_b[ch][None, :]),
            "vzero": np.zeros((TP, CH), np.float32),
        })
    return in_maps


def assemble_output(results, p_b):
    out = np.zeros((B, T, D), np.float32)
    for core in range(NCORES):
        out[core // HPC] += results[core]["y"].T
    out += p_b[None, None, :]
    return out


def _run(inputs, trace=False):
    from concourse.bass_utils import run_bass_kernel_spmd
    nc = build_program()
    args = {k: np.asarray(v, np.float32) for k, v in inputs.items()}
    p_b = args.pop("p_b")
    in_maps = make_in_maps(**args)
    res = run_bass_kernel_spmd(nc, in_maps, list(range(NCORES)), trace=trace)
    out = assemble_output(res.results, p_b)
    return out, res


def kernel(**inputs):
    out, _ = _run(inputs)
    return out
